# revision 31
# baseline (speedup 1.0000x reference)
"""Trainium2 Bass kernel: multi-edge-type GNN message passing.

out[t] = sum_l inv_sqrt_deg_l[t] * (sum_{e in type l, tgt_e = t} x[src_e]) @ W[l]

Strategy (8 NeuronCores, SPMD single program):
  - Host: per edge type, sort edges by target node; compute per-edge
    normalizer inv_sqrt_deg_l[tgt_e]; split target-node space into
    128-row tiles, assign a contiguous range of node tiles to each core
    (node/edge co-sharding => no collectives; outputs concatenate).
    Edges are split by source-node half (src < 32768 vs >=) so gathers
    can use the int16-indexed dma_gather fast path. Per (half, tile)
    ALL three edge types are packed into one padded chunk block (type
    boundaries fall mid-chunk and differ per core; per-(chunk, type)
    "units" carry type-masked one-hot metadata), which cuts descriptor
    padding vs per-(type, half, tile) chunk blocks.
  - Device, per gather instruction (up to GMAX 128-edge chunks):
      * one big dma_gather streams x[src_e] rows (bf16, 256B each) into
        an SBUF ring tile [128, GMAX, 128]; edge e of chunk k lands at
        partition e, free block k. 1024 idxs per instruction is the
        SWDGE descriptor-ring cap. Gathers round-robin over 4 SWDGE
        queues: queue q runs on Q7 cpu pair (2q, 2q+1), so 4 queues
        generate descriptors concurrently (~3.1us vs ~8.7us effective
        per 1024-row gather) -- Q7 descriptor generation, not HBM
        bandwidth, is the bottleneck of indexed gathers on TRN2.
      * batched one-hot build per 32 units: ONE broadcast DVE
        tensor_tensor is_equal makes O[e, u, t] = (tgt_u[e]==t) in bf16
        (tgt_u is -1 for slots outside the unit's type)
  - Device, per (node tile j, type l):  [tiles processed largest-first
    so the post-last-gather tail is short]
      * per unit: matmul-accumulate S^T[d,t] += xg_chunk.T @ O_u into
        PSUM (segment-sum on the PE, bf16 operands, f32 accumulation)
      * S^T -> bf16 SBUF, then S^T.T @ W[l] -> per-type output PSUM;
        finally out[t,:] = sum_l inv_l[t] * opsum_l[t,:] on DVE with
        per-partition f32 inv columns (inv_sqrt_deg is per (type,
        target), so it factors out of the edge sum)
  - Degree counts, rsqrt, sorting are index-side host preprocessing;
    all embedding data movement and FLOPs run on device.
"""

import numpy as np
import ml_dtypes
from contextlib import ExitStack

import concourse.bass as bass
import concourse.tile as tile
from concourse import bacc, mybir
from concourse.bass_utils import run_bass_kernel_spmd

P = 128
D = 128
N_CORES = 8
SPLIT = 32768          # int16 table split
GMAX = 8               # chunks per dma_gather (1024 idxs = SWDGE ring cap)
OHG = 32               # units per batched one-hot build
NQUEUES = 4            # SWDGE queues; each runs on its own Q7 cpu pair

F32 = mybir.dt.float32
BF16 = mybir.dt.bfloat16
I16 = mybir.dt.int16
BF16_NP = ml_dtypes.bfloat16

# test.py pokes this to get at profiling results of the last run
last_run_results = None


class Layout:
    """Uniform (core-independent) chunk/unit/gather layout."""

    def __init__(self, n_nodes, L, J):
        self.n_nodes = n_nodes
        self.L = L
        self.J = J
        self.C2 = np.zeros((2, J), dtype=np.int64)    # chunks per (h, j) block
        self.lo = np.zeros((2, J, L), dtype=np.int64)  # unit chunk ranges
        self.hi = np.zeros((2, J, L), dtype=np.int64)
        self.block_base = {}      # (h, j) -> first chunk gid
        self.units = []           # u -> (chunk gid, h, j, l, k) or dummy
        self.dummy = set()        # unit indices that are all-masked
        self.tile_units = None    # [l][j] -> list of unit indices
        self.chunk_meta = {}      # chunk gid -> (gather index, slot)
        self.gathers = []         # (h, first chunk gid, nch)
        self.ohgroups = []        # (first unit, n units)
        self.oh_meta = {}         # unit -> (group index, slot)
        self.NCH = 0
        self.NMM = 0

    def finalize(self):
        L, J = self.L, self.J
        gid = 0
        for h in range(2):
            h0 = gid
            for j in self.jorder:
                self.block_base[(h, j)] = gid
                gid += int(self.C2[h, j])
            for c0 in range(h0, gid, GMAX):
                nch = min(GMAX, gid - c0)
                gi = len(self.gathers)
                self.gathers.append((h, c0, nch))
                for k in range(nch):
                    self.chunk_meta[c0 + k] = (gi, k)
        self.NCH = gid

        self.tile_units = [[[] for _ in range(J)] for _ in range(L)]
        for h in range(2):
            u0 = len(self.units)
            for j in self.jorder:
                for l in range(L):
                    for k in range(int(self.lo[h, j, l]), int(self.hi[h, j, l])):
                        u = len(self.units)
                        self.units.append((self.block_base[(h, j)] + k, h, j, l, k))
                        self.tile_units[l][j].append(u)
                    if h == 1 and not self.tile_units[l][j]:
                        # no edges anywhere: dummy all-masked unit against a
                        # chunk of block (0, j), which is resident at tile j
                        u = len(self.units)
                        self.units.append((self.block_base[(0, j)], 1, j, l, -1))
                        self.dummy.add(u)
                        self.tile_units[l][j].append(u)
            for c0 in range(u0, len(self.units), OHG):
                n = min(OHG, len(self.units) - c0)
                oi = len(self.ohgroups)
                self.ohgroups.append((c0, n))
                for k in range(n):
                    self.oh_meta[c0 + k] = (oi, k)
        self.NMM = len(self.units)


def _preprocess(adjacency, n_nodes, n_cores):
    """Sort/shard edges. Returns (layout, per_core (idx16, tgtu, invu))."""
    L = adjacency.shape[0]
    tiles_total = -(-n_nodes // P)
    J = -(-tiles_total // n_cores)

    per_type = []
    inv_per_type = []
    for l in range(L):
        src = np.asarray(adjacency[l, :, 0], dtype=np.int64)
        tgt = np.asarray(adjacency[l, :, 1], dtype=np.int64)
        deg = np.bincount(tgt, minlength=n_nodes)
        inv = (1.0 / np.sqrt(np.maximum(deg, 1.0))).astype(np.float32)
        inv_per_type.append(inv)
        order = np.argsort(tgt, kind="stable")
        srcs = src[order]
        tgts = tgt[order]
        inve = inv[tgts]
        bounds = np.searchsorted(tgts, np.arange(tiles_total + 1) * P)
        per_type.append((srcs, tgts, inve, bounds))

    # per (core, l, h, j): edge lists split by src half, sorted by src
    edges = {}
    lay = Layout(n_nodes, L, J)
    cnt = np.zeros((n_cores, 2, J, L), dtype=np.int64)
    for l in range(L):
        srcs, tgts, inve, bounds = per_type[l]
        for c in range(n_cores):
            for j in range(J):
                t = c * J + j
                if t >= tiles_total:
                    continue
                lo, hi = int(bounds[t]), int(bounds[t + 1])
                s = srcs[lo:hi]
                tl = (tgts[lo:hi] - t * P).astype(np.float32)
                iv = inve[lo:hi]
                o = np.argsort(s, kind="stable")
                s, tl, iv = s[o], tl[o], iv[o]
                cut = int(np.searchsorted(s, SPLIT))
                edges[(c, l, 0, j)] = (s[:cut], tl[:cut], iv[:cut])
                edges[(c, l, 1, j)] = (s[cut:] - SPLIT, tl[cut:], iv[cut:])
                cnt[c, 0, j, l] = cut
                cnt[c, 1, j, l] = len(s) - cut

    # block sizes (max over cores) and per-type union chunk ranges
    for h in range(2):
        for j in range(J):
            tot = cnt[:, h, j, :].sum(axis=1)          # per core
            lay.C2[h, j] = -(-int(tot.max()) // P)
            if h == 0 and lay.C2[h, j] == 0:
                lay.C2[h, j] = 1
            start = np.concatenate(
                [np.zeros((n_cores, 1), np.int64),
                 np.cumsum(cnt[:, h, j, :], axis=1)], axis=1
            )
            for l in range(L):
                have = cnt[:, h, j, l] > 0
                if not have.any():
                    lay.lo[h, j, l] = lay.hi[h, j, l] = 0
                    continue
                s_ = start[have, l]
                e_ = start[have, l + 1]
                lay.lo[h, j, l] = int(s_.min()) // P
                lay.hi[h, j, l] = -(-int(e_.max()) // P)
    # process big tiles first so the tail (after the last gather) is short
    lay.jorder = [int(j) for j in np.argsort(-(lay.C2[0] + lay.C2[1]), kind="stable")]
    lay.finalize()

    NCH, NMM = lay.NCH, lay.NMM
    # per-node inv_sqrt_deg per type, for the end-stage column scaling
    inv_nodes = np.stack([pt_inv for pt_inv in inv_per_type])  # [L, n_nodes]
    empty = (np.zeros(0, np.int64), np.zeros(0, np.float32), np.zeros(0, np.float32))
    per_core = []
    for c in range(n_cores):
        idx16 = np.zeros((128, NCH * 8), np.int16)
        tgtu = np.full((P, NMM), -1.0, np.float32)
        invcol = np.zeros((P, L * J), np.float32)
        for l in range(L):
            for j in range(J):
                n0 = (c * J + j) * P
                n1 = min(n0 + P, n_nodes)
                if n1 > n0:
                    invcol[:n1 - n0, l * J + j] = inv_nodes[l, n0:n1]
        blocks = {}   # (h, j) -> (tl_all, iv_all, type_all)
        for h in range(2):
            for j in range(J):
                npad = int(lay.C2[h, j]) * P
                s_all = np.zeros(npad, np.int64)
                tl_all = np.full(npad, -1.0, np.float32)
                iv_all = np.zeros(npad, np.float32)
                ty_all = np.full(npad, -1, np.int64)
                pos = 0
                for l in range(L):
                    s, tl, iv = edges.get((c, l, h, j), empty)
                    n = len(s)
                    s_all[pos:pos + n] = s
                    tl_all[pos:pos + n] = tl
                    iv_all[pos:pos + n] = iv
                    ty_all[pos:pos + n] = l
                    pos += n
                blocks[(h, j)] = (tl_all, iv_all, ty_all)
                base = lay.block_base[(h, j)]
                for k in range(int(lay.C2[h, j])):
                    w = s_all[k * P:(k + 1) * P].astype(np.int16)
                    idx16[:, (base + k) * 8:(base + k + 1) * 8] = np.tile(
                        w.reshape(8, 16).T, (8, 1)
                    )
        for u, (gidc, h, j, l, k) in enumerate(lay.units):
            if u in lay.dummy:
                continue
            tl_all, iv_all, ty_all = blocks[(h, j)]
            sl = slice(k * P, (k + 1) * P)
            mask = ty_all[sl] == l
            tgtu[:, u] = np.where(mask, tl_all[sl], -1.0)
        per_core.append((idx16, tgtu, invcol))
    return lay, per_core


def _build_program(lay):
    # meta layout (bf16): [0,NMM) tgt_u | iota P | W L*D; invc is f32
    L, J = lay.L, lay.J
    NCH, NMM = lay.NCH, lay.NMM
    n_nodes = lay.n_nodes
    M = NMM + P + L * D
    nc = bacc.Bacc("TRN2", num_swdge_queues=NQUEUES)
    emb = nc.declare_dram_parameter("emb", [n_nodes, D], BF16, isOutput=False)
    idx_d = nc.declare_dram_parameter("idx16", [128, NCH * 8], I16, isOutput=False)
    meta_d = nc.declare_dram_parameter("meta", [P, M], BF16, isOutput=False)
    invc_d = nc.declare_dram_parameter("invc", [P, L * J], F32, isOutput=False)
    out_d = nc.declare_dram_parameter("out", [J * P, D], F32, isOutput=True)

    with tile.TileContext(nc) as tc, ExitStack() as ctx:
        const = ctx.enter_context(tc.tile_pool(name="const", bufs=1))
        xgp = [
            ctx.enter_context(tc.tile_pool(name=f"xg{h}", bufs=12)) for h in range(2)
        ]
        ohp = ctx.enter_context(tc.tile_pool(name="oh", bufs=6))
        stp = ctx.enter_context(tc.tile_pool(name="stsb", bufs=3))
        outp = ctx.enter_context(tc.tile_pool(name="osb", bufs=3))
        finp = ctx.enter_context(tc.tile_pool(name="fin", bufs=4))
        psum1 = ctx.enter_context(tc.tile_pool(name="ps1", bufs=2, space="PSUM"))
        psum2 = ctx.enter_context(tc.tile_pool(name="ps2", bufs=6, space="PSUM"))

        idx_sb = const.tile([128, NCH * 8], I16)
        nc.sync.dma_start(idx_sb[:], idx_d[:])
        meta_sb = const.tile([P, M], BF16)
        nc.scalar.dma_start(meta_sb[:], meta_d[:])
        invc_sb = const.tile([P, L * J], F32)
        nc.scalar.dma_start(invc_sb[:], invc_d[:])

        iota_ap = meta_sb[:, NMM:NMM + P]

        gbuf = [None] * len(lay.gathers)   # gi -> xg tile
        obuf = [None] * len(lay.ohgroups)  # oi -> oh tile

        def issue_gather(gi):
            h, g0, nch = lay.gathers[gi]
            xg = xgp[h].tile([P, GMAX, D], BF16, tag=f"xg{h}")
            in_ap = emb[:SPLIT, :] if h == 0 else emb[SPLIT:, :]
            nc.gpsimd.dma_gather(
                out_ap=xg[:, :nch, :],
                in_ap=in_ap,
                idxs_ap=idx_sb[:, g0 * 8:(g0 + nch) * 8],
                num_idxs=nch * P,
                num_idxs_reg=nch * P,
                elem_size=D,
                queue_num=gi % NQUEUES,
            )
            gbuf[gi] = xg

        def issue_ohgroup(oi):
            u0, n = lay.ohgroups[oi]
            oh = ohp.tile([P, OHG, P], BF16, tag="oh")
            nc.vector.tensor_tensor(
                out=oh[:, :n, :],
                in0=iota_ap.unsqueeze(1).broadcast_to([P, n, P]),
                in1=meta_sb[:, u0:u0 + n].unsqueeze(2).broadcast_to([P, n, P]),
                op=mybir.AluOpType.is_equal,
            )
            obuf[oi] = oh

        for j in lay.jorder:
            # make sure every gather/one-hot feeding this node tile is issued
            for l in range(L):
                for u in lay.tile_units[l][j]:
                    gi, _ = lay.chunk_meta[lay.units[u][0]]
                    if gbuf[gi] is None:
                        issue_gather(gi)
                    oi, _ = lay.oh_meta[u]
                    if obuf[oi] is None:
                        issue_ohgroup(oi)
            opsums = []
            for l in range(L):
                us = lay.tile_units[l][j]
                st_ps = psum1.tile([P, P], F32, tag="st")
                for k, u in enumerate(us):
                    gi, slot = lay.chunk_meta[lay.units[u][0]]
                    oi, oslot = lay.oh_meta[u]
                    nc.tensor.matmul(
                        out=st_ps[:],
                        lhsT=gbuf[gi][:, slot, :],
                        rhs=obuf[oi][:, oslot, :],
                        start=(k == 0),
                        stop=(k == len(us) - 1),
                    )
                st_sb = stp.tile([P, P], BF16, tag="stsb")
                nc.scalar.copy(st_sb[:], st_ps[:])
                opsum = psum2.tile([P, D], F32, tag="opsum")
                nc.tensor.matmul(
                    out=opsum[:],
                    lhsT=st_sb[:],
                    rhs=meta_sb[:, NMM + P + l * D:NMM + P + (l + 1) * D],
                    start=True,
                    stop=True,
                )
                opsums.append(opsum)
            # out[t, :] = sum_l inv_l[t] * opsum_l[t, :]  (inv applied in f32)
            t1 = finp.tile([P, D], F32, tag="fin")
            nc.vector.tensor_scalar(
                out=t1[:], in0=opsums[0][:],
                scalar1=invc_sb[:, 0 * J + j:0 * J + j + 1], scalar2=None,
                op0=mybir.AluOpType.mult,
            )
            t2 = finp.tile([P, D], F32, tag="fin")
            nc.vector.scalar_tensor_tensor(
                out=t2[:], in0=opsums[1][:],
                scalar=invc_sb[:, 1 * J + j:1 * J + j + 1], in1=t1[:],
                op0=mybir.AluOpType.mult, op1=mybir.AluOpType.add,
            )
            osb = outp.tile([P, D], F32, tag="osb")
            nc.vector.scalar_tensor_tensor(
                out=osb[:], in0=opsums[2][:],
                scalar=invc_sb[:, 2 * J + j:2 * J + j + 1], in1=t2[:],
                op0=mybir.AluOpType.mult, op1=mybir.AluOpType.add,
            )
            nc.sync.dma_start(out_d[j * P:(j + 1) * P, :], osb[:])
    nc.compile()
    return nc


def _run(node_embeddings, adjacency, W, n_cores=N_CORES, **run_kwargs):
    global last_run_results
    node_embeddings = np.ascontiguousarray(
        np.asarray(node_embeddings, dtype=np.float32)
    )
    adjacency = np.asarray(adjacency, dtype=np.int32)
    W = np.asarray(W, dtype=np.float32)
    n_nodes = node_embeddings.shape[0]
    L = adjacency.shape[0]

    lay, per_core = _preprocess(adjacency, n_nodes, n_cores)
    nc = _build_program(lay)

    emb16 = np.ascontiguousarray(node_embeddings.astype(BF16_NP))
    w_cat = np.concatenate([W[l] for l in range(L)], axis=1)
    iotaf = np.tile(np.arange(P, dtype=np.float32), (P, 1))
    in_maps = [
        dict(
            emb=emb16,
            idx16=idx16,
            meta=np.ascontiguousarray(
                np.concatenate([tg, iotaf, w_cat], axis=1).astype(BF16_NP)
            ),
            invc=np.ascontiguousarray(invcol),
        )
        for (idx16, tg, invcol) in per_core
    ]
    res = run_bass_kernel_spmd(nc, in_maps, core_ids=list(range(n_cores)), **run_kwargs)
    last_run_results = res
    outs = [res.results[c]["out"] for c in range(n_cores)]
    full = np.concatenate(outs, axis=0)[:n_nodes]
    return np.ascontiguousarray(full, dtype=np.float32)


def kernel(node_embeddings, adjacency, W):
    return _run(node_embeddings, adjacency, W)
